# revision 1
# baseline (speedup 1.0000x reference)
"""Trainium2 Bass kernel for 2D erosion (3x3 sliding-window min) on
x: (8, 4, 1024, 1024) f32, borders padded with +1e9 (pad never wins).

Strategy: pure data parallel over the 32 (b, c) images -> 4 images per core.
Device compute runs in bf16 (harness gate is rel_err < 2e-2; bf16 rounding is
monotone so min commutes with it -> error <= 2^-9): halves DMA bytes and
doubles DVE throughput (2x_1p mode needs 2-byte dtype + unit-stride innermost
dims, which the op formulation below maintains).

Per-core DRAM input is a (4101, 1024) bf16 stack: 4 images with one 1e9 pad
row between/around. Per image, one overlapping load puts DRAM rows
8p-1 .. 8p+8 (10 rows, 20KB) in partition p — the +-1 halo rows ride along,
so the vertical pass needs no cross-partition traffic and no separate halo
DMA.

Vertical (H) pass, 3 DVE ops via pair-sharing (x rows indexed 0..9 in-tile,
v[r] = min over x[r .. r+2]):
    s[k]      = min(x[2k+1], x[2k+2])   k=0..3
    v[even r] = min(x[r], s[r/2])
    v[odd r]  = min(s[(r-1)/2], x[r+2])
All operands are 3D APs with unit-stride 1024-wide innermost dims -> 2x mode.

Horizontal (W) pass, shift formulation (unit strides, 2x mode), with the
final combine offloaded to GPSIMD to balance engine load:
    t[j] = min(v[j], v[j+1])            (DVE)
    o[j] = min(t[j-1], t[j])            (GPSIMD)
Row-boundary columns (first/last of each 1024-wide row) shrink to a 2-tap
window = the adjacent t value; one tiny strided copy per image fixes both
edges of all 8 rows.

Loads ride the SP HWDGE ring, stores the ACT ring, so they don't queue
behind each other.
"""

import numpy as np
import ml_dtypes

import concourse.bass as bass
import concourse.bacc as bacc
import concourse.mybir as mybir
from concourse.tile import TileContext
from concourse.bass_utils import run_bass_kernel_spmd

N_CORES = 8
B, C, H, W = 8, 4, 1024, 1024
IMGS = B * C // N_CORES  # images per core = 4
P = 128                  # SBUF partitions
R = H // P               # image rows per partition = 8
RL = R + 2               # loaded rows per partition (incl +-1 halo)
F = R * W                # free-dim elements per partition = 8192
PAD = 1.0e9
XROWS = IMGS * (H + 1) + 1  # padded per-core input rows = 4101
BF16 = mybir.dt.bfloat16
MIN = mybir.AluOpType.min
NP_BF16 = ml_dtypes.bfloat16

_NC_CACHE = {}


LOOP_BODY_REPS = 4  # reps unrolled inside the hardware loop body


def _emit_image(nc, pools, i):
    """Emit load -> H pass -> W pass -> store for image i."""
    xpool, spool, vpool, tpool, opool = pools
    x, y = nc._x, nc._y
    base = 1 + i * (H + 1)  # first image row in the padded stack

    xt = xpool.tile([P, RL * W], BF16)
    # overlapping load: partition p <- DRAM rows base-1+8p .. base+8+8p
    src = bass.AP(x, (base - 1) * W, [[R * W, P], [1, RL * W]])
    nc.sync.dma_start(out=xt, in_=src)

    # ---- H pass: v[r] = min(x[r], x[r+1], x[r+2]) (tile rows), computed as
    # two flat 1D row-shift mins. On HW only flat unit-stride APs hit the
    # DVE 2x bf16 mode; row-strided 3D APs fall back to 1x.
    a = spool.tile([P, (RL - 1) * W], BF16)  # a[q] = min(x[q], x[q+W])
    v = vpool.tile([P, F], BF16)
    nc.vector.tensor_tensor(
        out=a, in0=xt[:, 0 : (RL - 1) * W], in1=xt[:, W : RL * W], op=MIN
    )
    nc.vector.tensor_tensor(
        out=v, in0=a[:, 0:F], in1=a[:, W : (RL - 1) * W], op=MIN
    )

    # ---- W pass: o[j] = min(v[j-1], v[j], v[j+1]) within rows ----
    t = tpool.tile([P, F], BF16)  # t[0..F-2] valid
    nc.vector.tensor_tensor(
        out=t[:, 0 : F - 1], in0=v[:, 0 : F - 1], in1=v[:, 1:F], op=MIN
    )
    o = opool.tile([P, F], BF16)
    nc.vector.tensor_tensor(
        out=o[:, 1 : F - 1], in0=t[:, 0 : F - 2], in1=t[:, 1 : F - 1], op=MIN
    )
    # per-row first/last column: window shrinks to 2 taps = t value
    orr = o.rearrange("p (r w) -> p r w", r=R)
    tr = t.rearrange("p (r w) -> p r w", r=R)
    nc.vector.tensor_copy(
        out=orr[:, :, 0 : W : W - 1], in_=tr[:, :, 0 : W - 1 : W - 2]
    )

    # store on the ACT HWDGE ring (parallel to SP loads)
    ym = y[i].rearrange("(p r) w -> p (r w)", p=P)
    nc.scalar.dma_start(out=ym, in_=o)


def _build_nc(reps=1):
    nc = bacc.Bacc()
    nc._x = nc.dram_tensor("x", (XROWS, W), BF16, kind="ExternalInput")
    nc._y = nc.dram_tensor("y", (IMGS, H, W), BF16, kind="ExternalOutput")

    with TileContext(nc) as tc:
        with (
            tc.tile_pool(name="xp", bufs=3) as xpool,
            tc.tile_pool(name="sp", bufs=2) as spool,
            tc.tile_pool(name="vp", bufs=2) as vpool,
            tc.tile_pool(name="tp", bufs=2) as tpool,
            tc.tile_pool(name="op", bufs=2) as opool,
        ):
            pools = (xpool, spool, vpool, tpool, opool)
            if reps <= 48:
                for i in [im for _ in range(reps) for im in range(IMGS)]:
                    _emit_image(nc, pools, i)
            else:
                # timing mode: hardware loop keeps the NEFF compact so reps
                # can be large enough to swamp host/tunnel timing noise
                n_iter, rem = divmod(reps, LOOP_BODY_REPS)
                with tc.For_i(0, n_iter, 1):
                    for i in [
                        im for _ in range(LOOP_BODY_REPS) for im in range(IMGS)
                    ]:
                        _emit_image(nc, pools, i)
                for i in [im for _ in range(rem) for im in range(IMGS)]:
                    _emit_image(nc, pools, i)

    nc.finalize()
    return nc


def _get_nc(reps=1):
    if reps not in _NC_CACHE:
        _NC_CACHE[reps] = _build_nc(reps)
    return _NC_CACHE[reps]


def _to_bf16(x):
    """f32 -> bf16 with round-to-nearest-even (vectorized bit trick)."""
    u = np.ascontiguousarray(x, dtype=np.float32).view(np.uint32)
    r = ((u + 0x7FFF + ((u >> 16) & 1)) >> 16).astype(np.uint16)
    return r.view(NP_BF16)


def _pad_shard(shard_bf16):
    """(IMGS, H, W) bf16 -> (XROWS, W) bf16 with 1e9 pad rows between/around."""
    out = np.full((XROWS, W), PAD, dtype=NP_BF16)
    for i in range(IMGS):
        base = 1 + i * (H + 1)
        out[base : base + H] = shard_bf16[i]
    return out


def kernel(x: np.ndarray, _reps: int = 1):
    assert x.shape == (B, C, H, W)
    xb = _to_bf16(x).reshape(N_CORES, IMGS, H, W)
    nc = _get_nc(_reps)
    in_maps = [{"x": _pad_shard(xb[k])} for k in range(N_CORES)]
    res = run_bass_kernel_spmd(nc, in_maps, core_ids=list(range(N_CORES)))
    out16 = np.stack([r["y"] for r in res.results], axis=0)
    # bf16 -> f32 upcast via bit shift
    out = (out16.view(np.uint16).astype(np.uint32) << 16).view(np.float32)
    return out.reshape(B, C, H, W)



# revision 8
# speedup vs baseline: 1.4613x; 1.4613x over previous
"""Trainium2 Bass kernel for 2D erosion (3x3 sliding-window min) on
x: (8, 4, 1024, 1024) f32, borders padded with +1e9 (pad never wins).

Strategy: pure data parallel over the 32 (b, c) images -> 4 images per core.
Device compute runs in bf16 (harness gate is rel_err < 2e-2; bf16 rounding is
monotone so min commutes with it -> error <= 2^-9): halves DMA bytes and
enables the DVE 2x bf16 mode (2 elem/cycle/lane, tensor_tensor's fastest).

Work floor: a 3-tap sliding min costs 1.5 binary ops/elem per axis via
pair-sharing (s[m]=min(x[2m+1],x[2m+2]) feeds both v[2m] and v[2m+1]),
so the separable 3x3 is 3.0 ops/elem -- vs 4.0 for the naive shift chain.
Vertical sharing needs row-strided (3D) operand APs: measured on HW these
run at full 2x (4104-elem op, both inputs strided: 2383ns ~= flat).
Horizontal sharing needs even/odd column planes, so input is staged
column-deinterleaved, plane-major, with one PAD slot per 513-wide row
(EC row = [c0,c2,..,c1022,PAD], OC row = [PAD,c1,..,c1023]); the pad
slots make every row-boundary and image-edge case fall out of the flat
min ops with zero fixup instructions.

Layout: partition p = 32*i + j owns 32 output rows (32j..32j+31) of image
i; its input window is 34 rows (1 halo row each side, PAD rows at image
borders), staged as two 18-row half-tiles (window rows 0-17 / 16-33;
1.125x input halo) of [EC plane 18x513 | OC plane 18x513].

Per 8-row chunk (window rows r0..r0+9 inside a half-tile), 9 DVE ops:
  s_P  = min(xP[r0+1..r0+8:2], xP[r0+2..r0+9:2])   P in {EC,OC}  (2052 ea)
  veP  = min(xP[r0..r0+7:2],  s_P)   -> even output rows          (2052 ea)
  voP  = min(s_P, xP[r0+3..r0+10:2]) -> odd output rows           (2052 ea)
  (the four v ops write one v tile: [EC_e | OC_e | EC_o | OC_o])
  q    = min(EC[s], OC[s+1])         both row-groups via 3D AP    (4102)
  o_ec = min(OC[s], q[s])    -> even cols                         (4102)
  o_oc = min(q[s-1], EC[s])  -> odd cols                          (4102)
Output tile [o_ec_e | o_oc_e | o_ec_o | o_oc_o] (4x2052, pad slots
carried to DRAM and dropped on host). Loads ride the SP HWDGE ring,
stores the ACT ring. Modeled steady state: DMA ~50us, DVE ~53us.
"""

import numpy as np
import ml_dtypes

import concourse.bass as bass
import concourse.bacc as bacc
import concourse.mybir as mybir
from concourse.tile import TileContext
from concourse.bass_utils import run_bass_kernel_spmd

N_CORES = 8
B, C, H, W = 8, 4, 1024, 1024
IMGS = B * C // N_CORES  # images per core = 4
P = 128                  # SBUF partitions
S = W // 2 + 1           # column-plane row width incl pad slot = 513
HT = 18                  # rows per half-tile (16 + 2 halo)
PL = HT * S              # elems per plane per half-tile = 9234
G = 4 * S                # 4-row group block = 2052
PAD = 1.0e9
BF16 = mybir.dt.bfloat16
MIN = mybir.AluOpType.min
NP_BF16 = ml_dtypes.bfloat16

_NC_CACHE = {}

LOOP_BODY_REPS = 4  # reps unrolled inside the hardware loop body


def _emit_rep(nc, pools):
    """Emit loads -> 4 chunks (shared-pair vert+horiz min, store)."""
    xpool, spool, vpool, qpool, opool = pools
    x, y = nc._x, nc._y

    # two half-tile loads on the SP HWDGE ring
    xts = []
    for h in range(2):
        xt = xpool.tile([P, 2 * PL], BF16)
        src = bass.AP(x, h * 2 * PL, [[4 * PL, P], [1, 2 * PL]])
        nc.sync.dma_start(out=xt, in_=src)
        xts.append(xt)

    for c in range(4):
        xt = xts[c // 2]
        r0 = 8 * (c % 2)  # chunk's first window row within the half-tile
        # per-plane 3D row views: [P, 18, S] at plane offset
        xEC = xt[:, 0:PL].rearrange("p (r w) -> p r w", w=S)
        xOC = xt[:, PL : 2 * PL].rearrange("p (r w) -> p r w", w=S)

        # vertical pass with pair-sharing (row-strided 3D inputs, flat outs)
        s = spool.tile([P, 2 * G], BF16)  # [s_EC | s_OC]
        v = vpool.tile([P, 4 * G], BF16)  # [EC_e | OC_e | EC_o | OC_o]
        for pi, xP in enumerate((xEC, xOC)):
            sP = s[:, pi * G : (pi + 1) * G]
            nc.vector.tensor_tensor(
                out=sP, in0=xP[:, r0 + 1 : r0 + 9 : 2, :],
                in1=xP[:, r0 + 2 : r0 + 10 : 2, :], op=MIN,
            )
            nc.vector.tensor_tensor(
                out=v[:, pi * G : (pi + 1) * G],
                in0=xP[:, r0 : r0 + 8 : 2, :], in1=sP, op=MIN,
            )
            nc.vector.tensor_tensor(
                out=v[:, (2 + pi) * G : (3 + pi) * G],
                in0=sP, in1=xP[:, r0 + 3 : r0 + 10 : 2, :], op=MIN,
            )

        # horizontal pass with pair-sharing, per row-group, all-flat APs.
        # q[s] = min(EC[s], OC[s+1]) = min(v[2s], v[2s+1]); pad slots make
        # row boundaries and image edges self-correcting.
        q = qpool.tile([P, 2 * G], BF16)  # [q_e | q_o]
        o = opool.tile([P, 4 * G], BF16)  # [o_ec_e | o_oc_e | o_ec_o | o_oc_o]
        for g in range(2):
            vg, qg, og = 2 * g * G, g * G, 2 * g * G  # group base offsets
            nc.vector.tensor_tensor(
                out=q[:, qg : qg + G - 1],
                in0=v[:, vg : vg + G - 1],
                in1=v[:, vg + G + 1 : vg + 2 * G], op=MIN,
            )
            nc.vector.tensor_tensor(
                out=o[:, og : og + G - 1],           # o_ec_g: cols 2s
                in0=v[:, vg + G : vg + 2 * G - 1],   # OC[s]
                in1=q[:, qg : qg + G - 1], op=MIN,
            )
            nc.vector.tensor_tensor(
                out=o[:, og + G + 1 : og + 2 * G],   # o_oc_g: cols 2s+1
                in0=q[:, qg : qg + G - 1],           # q[s]
                in1=v[:, vg + 1 : vg + G], op=MIN,   # EC[s+1]
            )

        # store on the ACT HWDGE ring (parallel to SP loads)
        nc.scalar.dma_start(out=y[:, c * 4 * G : (c + 1) * 4 * G], in_=o)


def _build_nc(reps=1):
    nc = bacc.Bacc()
    nc._x = nc.dram_tensor("x", (P, 4 * PL), BF16, kind="ExternalInput")
    nc._y = nc.dram_tensor("y", (P, 16 * G), BF16, kind="ExternalOutput")

    with TileContext(nc) as tc:
        with (
            tc.tile_pool(name="xp", bufs=3) as xpool,
            tc.tile_pool(name="sp", bufs=1) as spool,
            tc.tile_pool(name="vp", bufs=1) as vpool,
            tc.tile_pool(name="qp", bufs=1) as qpool,
            tc.tile_pool(name="op", bufs=2) as opool,
        ):
            pools = (xpool, spool, vpool, qpool, opool)
            if reps <= 48:
                for _ in range(reps):
                    _emit_rep(nc, pools)
            else:
                # timing mode: hardware loop keeps the NEFF compact so reps
                # can be large enough to swamp host/tunnel timing noise
                n_iter, rem = divmod(reps, LOOP_BODY_REPS)
                with tc.For_i(0, n_iter, 1):
                    for _ in range(LOOP_BODY_REPS):
                        _emit_rep(nc, pools)
                for _ in range(rem):
                    _emit_rep(nc, pools)

    nc.finalize()
    return nc


def _get_nc(reps=1):
    if reps not in _NC_CACHE:
        _NC_CACHE[reps] = _build_nc(reps)
    return _NC_CACHE[reps]


def _to_bf16(x):
    """f32 -> bf16 with round-to-nearest-even (vectorized bit trick)."""
    u = np.ascontiguousarray(x, dtype=np.float32).view(np.uint32)
    r = ((u + 0x7FFF + ((u >> 16) & 1)) >> 16).astype(np.uint16)
    return r.view(NP_BF16)


def _stage_shard(shard_bf16):
    """(IMGS, H, W) bf16 -> (128, 4*PL) staged column-plane half-tiles."""
    out = np.empty((P, 4 * PL), dtype=NP_BF16)
    padrow = np.full((1, W), PAD, dtype=NP_BF16)
    padcol = np.full((H + 2, 1), PAD, dtype=NP_BF16)
    for i in range(IMGS):
        pi = np.concatenate([padrow, shard_bf16[i], padrow], axis=0)  # (1026, W)
        ec = np.concatenate([pi[:, 0::2], padcol], axis=1)  # (1026, S)
        oc = np.concatenate([padcol, pi[:, 1::2]], axis=1)
        for h in range(2):
            # partition 32i+j, half h: window rows 32j+16h .. +18
            idx = (32 * np.arange(32)[:, None] + 16 * h
                   + np.arange(HT)[None, :])  # (32, 18)
            dst = out[32 * i : 32 * i + 32, 2 * PL * h : 2 * PL * (h + 1)]
            dst[:, :PL] = ec[idx].reshape(32, PL)
            dst[:, PL:] = oc[idx].reshape(32, PL)
    return out


def _unstage_out(y16):
    """(n_cores, 128, 16*G) bf16 -> (n_cores, IMGS, H, W) dropping pad slots."""
    n = y16.shape[0]
    y7 = y16.reshape(n, P, 4, 4, 4, S)  # [core, p, chunk, region, row, slot]
    out = np.empty((n, P, 4, 8, W), dtype=y16.dtype)  # [core, p, chunk, rowinchunk, col]
    out[:, :, :, 0::2, 0::2] = y7[:, :, :, 0, :, 0 : S - 1]  # e rows, even cols
    out[:, :, :, 0::2, 1::2] = y7[:, :, :, 1, :, 1:S]        # e rows, odd cols
    out[:, :, :, 1::2, 0::2] = y7[:, :, :, 2, :, 0 : S - 1]  # o rows, even cols
    out[:, :, :, 1::2, 1::2] = y7[:, :, :, 3, :, 1:S]        # o rows, odd cols
    return out.reshape(n, IMGS, H, W)


def kernel(x: np.ndarray, _reps: int = 1):
    assert x.shape == (B, C, H, W)
    xb = _to_bf16(x).reshape(N_CORES, IMGS, H, W)
    nc = _get_nc(_reps)
    in_maps = [{"x": _stage_shard(xb[k])} for k in range(N_CORES)]
    res = run_bass_kernel_spmd(nc, in_maps, core_ids=list(range(N_CORES)))
    out16 = _unstage_out(np.stack([r["y"] for r in res.results], axis=0))
    # bf16 -> f32 upcast via bit shift
    out = (out16.view(np.uint16).astype(np.uint32) << 16).view(np.float32)
    return out.reshape(B, C, H, W)
